# revision 27
# baseline (speedup 1.0000x reference)
"""Plackett-Luce listwise loss kernel for Trainium2 (Bass/Tile), 8-core data parallel.

Algorithm (per row of 32 items):
  loss_row = sum_k log(T_k) - sum_i s_i*valid_i, where T_k are the suffix sums
  of exp(s) over items sorted by (rank, position) (stable sort, padded last).
  Final: mean over rows with n>=2 of loss_row/n.

Host packs each item into one fp32 value
  V = (2047 - ((rank + 32*mask)*32 + pos))*2^14 + round((s*valid + 8)*2^10)
so the device receives a single [B,32] fp32 tensor (inverted key: valid items
carry bit 24, padded do not, and ascending sort puts padded first). Device:
Batcher odd-even merge sort ASCENDING per 32-item segment, int-convert + AND
0x1003FFF (score field q plus the validity bit), ACT exp with bias
-16392 = -8 - 2^24/1024 (valid -> exp(s) exactly, padded -> exp(~-16384) = 0),
gated prefix scan for the suffix sums, ACT ln with +1e-12 bias (padded lanes
scan to exactly 0 -> ln(1e-12) = C0, corrected per row), and two per-row
reductions: sum(ln T) and the int32 sum(u3) whose bits >=24 count valid items
and low bits hold the score sum. Two chains of j=128 segments/partition sort
in stage-lockstep (each chain's ACT copy-backs hide under the other's DVE
compare-exchanges); their decodes then run with pieces alternating between
chains so ACT and DVE dovetail in the tail. Each core reduces to a [128, 2]
partial; host sums and divides.
"""

import sys

for _p in ("/opt/trn_rl_repo", "/root/.axon_site/_ro/trn_rl_repo"):
    if _p not in sys.path:
        sys.path.insert(0, _p)

import numpy as np

P = 128
N = 32
NCORES = 8
B = 262144
B_CORE = B // NCORES  # 32768
CHAINS = [128, 128]  # segments/partition per chain (sums to 256)

# Batcher odd-even merge sort, n=32.
# (k, offset, per-segment pattern [[step,count],...], needs_precopy)
SORT_STAGES = [
    (1, 0, [[2, 16]], False),
    (2, 0, [[4, 8], [1, 2]], False),
    (1, 1, [[4, 8]], True),
    (4, 0, [[8, 4], [1, 4]], False),
    (2, 2, [[8, 4], [1, 2]], True),
    (1, 1, [[8, 4], [2, 3]], True),
    (8, 0, [[16, 2], [1, 8]], False),
    (4, 4, [[16, 2], [1, 4]], True),
    (2, 2, [[16, 2], [4, 3], [1, 2]], True),
    (1, 1, [[16, 2], [2, 7]], True),
    (16, 0, [[1, 16]], False),
    (8, 8, [[1, 8]], True),
    (4, 4, [[8, 3], [1, 4]], True),
    (2, 2, [[4, 7], [1, 2]], True),
    (1, 1, [[2, 15]], True),
]

SC_S = float(2 ** 10)
C0 = float(np.log(np.float32(1e-12)))  # HW Ln table value at the bias


def _pattern_ap(bass_mod, tile_ap, off, dims, j):
    """AP over a [P, j*32] tile selecting `dims` within each 32-item segment."""
    base = tile_ap
    pdim = base.ap[0]
    if dims[0][0] * dims[0][1] == N:
        free = [[dims[0][0], dims[0][1] * j]] + [list(d) for d in dims[1:]]
    else:
        free = [[N, j]] + [list(d) for d in dims]
    return bass_mod.AP(tensor=base.tensor, offset=base.offset + off, ap=[list(pdim)] + free)


def build_program(b_core=B_CORE, chains=CHAINS):
    import concourse.bass as bass
    import concourse.bacc as bacc
    import concourse.tile as tile
    from concourse import mybir

    assert sum(chains) == b_core // P

    nc = bacc.Bacc("TRN2")
    v_d = nc.dram_tensor("packed", [b_core, N], mybir.dt.float32, kind="ExternalInput")
    o_d = nc.dram_tensor("partial", [P, 2], mybir.dt.float32, kind="ExternalOutput")

    op = mybir.AluOpType
    act = mybir.ActivationFunctionType

    with tile.TileContext(nc) as tc:
        with (
            tc.tile_pool(name="singles", bufs=1) as singles,
            tc.tile_pool(name="stream", bufs=1) as stream,
            tc.tile_pool(name="deep", bufs=1) as deep,
        ):
            fmax = max(chains) * N
            _pre = {}

            def _preload(chains_):
                jj0 = 0
                for ci, jj in enumerate(chains_):
                    r0 = jj0 * P
                    ff = jj * N
                    v_a = deep.tile([P, ff], mybir.dt.float32, name=f"va{jj0}")
                    if ci == 0:
                        # first chain split across two queue slots: lands ~2x
                        # sooner, so the first sort stage starts earlier
                        hj = jj // 2
                        # issue on the ACT queue: it boots ~2us before Sync,
                        # so the first sort stage starts sooner
                        nc.scalar.dma_start(
                            out=v_a[:, 0:hj * N],
                            in_=v_d[r0:r0 + P * hj, :].rearrange(
                                "(p j) n -> p (j n)", p=P))
                        nc.scalar.dma_start(
                            out=v_a[:, hj * N:],
                            in_=v_d[r0 + P * hj:r0 + P * jj, :].rearrange(
                                "(p j) n -> p (j n)", p=P))
                    else:
                        nc.sync.dma_start(
                            out=v_a[:],
                            in_=v_d[r0:r0 + P * jj, :].rearrange("(p j) n -> p (j n)", p=P))
                    _pre[jj0] = v_a
                    jj0 += jj

            _preload(chains)
            gate = singles.tile([P, fmax], mybir.dt.float32)
            nc.vector.memset(gate[:], 1.0)
            g3 = gate[:].rearrange("p (j n) -> p j n", n=N)
            nc.vector.memset(g3[:, :, 0:1], 0.0)
            b_exp = singles.tile([P, 1], mybir.dt.float32)
            nc.vector.memset(b_exp[:], -16392.0)
            b_ln = singles.tile([P, 1], mybir.dt.float32)
            nc.vector.memset(b_ln[:], 1e-12)

            js = b_core // P
            pr_all = singles.tile([P, js], mybir.dt.float32)
            use_all = singles.tile([P, js], mybir.dt.float32)

            def load(j0, j):
                r0 = j0 * P
                f = j * N
                v_a = _pre[j0]
                v_b = deep.tile([P, f], mybir.dt.float32, name=f"vb{j0}")
                scratch = deep.tile([P, f // 2], mybir.dt.float32, name=f"sc{j0}")
                return [v_a, v_b, scratch, j, j0]

            def emit_stage(st, stage):
                (k, off, dims, precopy) = stage
                cur, oth, scratch, j, _ = st
                lo_i = _pattern_ap(bass, cur[:], off, dims, j)
                hi_i = _pattern_ap(bass, cur[:], off + k, dims, j)
                if precopy:
                    npair = j
                    for d in dims:
                        npair *= d[1]
                    sc = scratch[:, 0:npair]
                    nc.vector.tensor_tensor(out=sc, in0=lo_i, in1=hi_i, op=op.min)
                    nc.vector.tensor_tensor(out=hi_i, in0=lo_i, in1=hi_i, op=op.max)
                    nc.scalar.copy(out=lo_i, in_=sc)
                else:
                    lo_o = _pattern_ap(bass, oth[:], off, dims, j)
                    hi_o = _pattern_ap(bass, oth[:], off + k, dims, j)
                    nc.vector.tensor_tensor(out=lo_o, in0=lo_i, in1=hi_i, op=op.min)
                    nc.vector.tensor_tensor(out=hi_o, in0=lo_i, in1=hi_i, op=op.max)
                    st[0], st[1] = oth, cur

            def decode_pieces(st):
                """Decode steps for a sorted chain, yielded between the next
                chain's sort stages so DVE work interleaves and ACT overlaps.
                Elementwise work is chunked (32 segments at a time) so no
                single ACT op can stall the next chain's precopy copy-backs."""
                v_s = st[0]
                j, j0 = st[3], st[4]
                f = j * N
                ctx = {}
                CH = 128  # segments per chunk
                chunks = [(c, min(c + CH, j)) for c in range(0, j, CH)]

                def mk_convert(c0, c1):
                    def p():
                        if "vi" not in ctx:
                            ctx["vi"] = stream.tile([P, f], mybir.dt.int32, name=f"vi{j0}")
                        nc.scalar.copy(out=ctx["vi"][:, c0 * N:c1 * N],
                                       in_=v_s[:, c0 * N:c1 * N])
                    return p

                def mk_and(c0, c1):
                    def p():
                        vi = ctx["vi"]
                        sl = vi[:, c0 * N:c1 * N]
                        nc.vector.tensor_scalar(out=sl, in0=sl, scalar1=0x1003FFF,
                                                scalar2=None, op0=op.bitwise_and)
                    return p

                def mk_usum(c0, c1):
                    def p():
                        if "usum" not in ctx:
                            ctx["usum"] = singles.tile([P, j], mybir.dt.int32, name=f"usum{j0}")
                        with nc.allow_low_precision("int32 reduce exact"):
                            nc.vector.tensor_reduce(
                                out=ctx["usum"][:, c0:c1],
                                in_=ctx["vi"][:, c0 * N:c1 * N].rearrange(
                                    "p (j n) -> p j n", n=N),
                                axis=mybir.AxisListType.X, op=op.add)
                    return p

                def mk_exp(c0, c1):
                    def p():
                        if "e" not in ctx:
                            ctx["e"] = stream.tile([P, f], mybir.dt.float32, name=f"e{j0}")
                        nc.scalar.activation(out=ctx["e"][:, c0 * N:c1 * N],
                                             in_=ctx["vi"][:, c0 * N:c1 * N],
                                             func=act.Exp, bias=b_exp[:],
                                             scale=1.0 / SC_S)
                    return p

                def mk_scan(c0, c1):
                    def p():
                        if "t" not in ctx:
                            ctx["t"] = stream.tile([P, f], mybir.dt.float32, name=f"t{j0}")
                        fc = (c1 - c0) * N
                        nc.vector.tensor_tensor_scan(
                            out=ctx["t"][:, c0 * N:c1 * N], data0=gate[:, 0:fc],
                            data1=ctx["e"][:, c0 * N:c1 * N],
                            initial=0.0, op0=op.mult, op1=op.add)
                    return p

                def mk_ln(c0, c1):
                    def p():
                        sl = ctx["t"][:, c0 * N:c1 * N]
                        nc.scalar.activation(out=sl, in_=sl, func=act.Ln,
                                             bias=b_ln[:], scale=1.0)
                    return p

                def mk_lsum(c0, c1):
                    def p():
                        if "lsum" not in ctx:
                            ctx["lsum"] = singles.tile([P, j], mybir.dt.float32, name=f"lsum{j0}")
                        nc.vector.tensor_reduce(
                            out=ctx["lsum"][:, c0:c1],
                            in_=ctx["t"][:, c0 * N:c1 * N].rearrange(
                                "p (j n) -> p j n", n=N),
                            axis=mybir.AxisListType.X, op=op.add)
                    return p

                def p_weights():
                    # usum = sum(q) + 2^24*n, 0 <= sum(q) < 2^20
                    usum = ctx["usum"]
                    n_i = singles.tile([P, j], mybir.dt.int32)
                    nc.vector.tensor_scalar(out=n_i[:], in0=usum[:], scalar1=24,
                                            scalar2=None, op0=op.arith_shift_right)
                    sq_i = singles.tile([P, j], mybir.dt.int32)
                    nc.vector.tensor_scalar(out=sq_i[:], in0=usum[:],
                                            scalar1=0xFFFFFF, scalar2=None,
                                            op0=op.bitwise_and)
                    n_t = singles.tile([P, j], mybir.dt.float32)
                    nc.vector.tensor_copy(out=n_t[:], in_=n_i[:])
                    # m = svr - C0*npad = sq/1024 - 256 - C0*(32 - n)
                    svr = singles.tile([P, j], mybir.dt.float32)
                    with nc.allow_low_precision("values < 2^20, exact in fp32"):
                        nc.vector.tensor_scalar(out=svr[:], in0=sq_i[:],
                                                scalar1=1.0 / SC_S,
                                                scalar2=-256.0 + 32.0 * C0,
                                                op0=op.mult, op1=op.add)
                    m_t = singles.tile([P, j], mybir.dt.float32)
                    nc.vector.scalar_tensor_tensor(
                        out=m_t[:], in0=n_t[:], scalar=-C0, in1=svr[:],
                        op0=op.mult, op1=op.add)
                    ctx["m"] = m_t
                    # weight = (n>=2)/max(n,1); reciprocal on ACT
                    nmx = singles.tile([P, j], mybir.dt.float32)
                    nc.vector.tensor_scalar_max(nmx[:], n_t[:], 1.0)
                    wrec = singles.tile([P, j], mybir.dt.float32)
                    nc.vector.reciprocal(wrec[:], nmx[:])
                    use = use_all[:, j0:j0 + j]
                    nc.vector.tensor_single_scalar(out=use, in_=n_t[:], scalar=2.0,
                                                   op=op.is_ge)
                    w3 = singles.tile([P, j], mybir.dt.float32)
                    nc.vector.tensor_tensor(out=w3[:], in0=wrec[:], in1=use,
                                            op=op.mult)
                    ctx["w3"] = w3

                def p_final():
                    # pr = (lsum - m) * w3
                    d_t = singles.tile([P, j], mybir.dt.float32)
                    nc.vector.tensor_sub(d_t[:], ctx["lsum"][:], ctx["m"][:])
                    nc.vector.tensor_tensor(out=pr_all[:, j0:j0 + j], in0=d_t[:],
                                            in1=ctx["w3"][:], op=op.mult)

                pieces = []
                for c0, c1 in chunks:
                    pieces += [mk_convert(c0, c1), mk_and(c0, c1),
                               mk_usum(c0, c1), mk_exp(c0, c1),
                               mk_scan(c0, c1), mk_ln(c0, c1), mk_lsum(c0, c1)]
                pieces += [p_weights, p_final]
                return pieces

            # lockstep sorts of the two chains (ACT copy-backs of one chain
            # overlap the other's DVE stages), then their decodes with pieces
            # alternating so ACT and DVE dovetail in the tail
            sts = []
            j0 = 0
            for j in chains:
                sts.append(load(j0, j))
                j0 += j
            plists = None
            for si, stage in enumerate(SORT_STAGES):
                last = si == len(SORT_STAGES) - 1
                for ci, st in enumerate(sts):
                    emit_stage(st, stage)
                    if last and ci == 0:
                        plists = [decode_pieces(s) for s in sts]
                        plists[0].pop(0)()
            k = 0
            while any(plists):
                for pl in plists:
                    if pl:
                        pl.pop(0)()

            out_t = singles.tile([P, 2], mybir.dt.float32)
            nc.vector.tensor_reduce(out=out_t[:, 0:1], in_=pr_all[:],
                                    axis=mybir.AxisListType.X, op=op.add)
            nc.vector.tensor_reduce(out=out_t[:, 1:2], in_=use_all[:],
                                    axis=mybir.AxisListType.X, op=op.add)
            nc.sync.dma_start(out=o_d[:], in_=out_t[:])

    nc.finalize()
    return nc


_CACHED = {}


def _get_program():
    if "nc" not in _CACHED:
        _CACHED["nc"] = build_program()
    return _CACHED["nc"]


def _pack(scores, ranks, mask):
    scores = np.asarray(scores, dtype=np.float32)
    ranks = np.asarray(ranks)
    mask = np.asarray(mask).astype(bool)
    key = (ranks.astype(np.int32) + 32 * mask.astype(np.int32)) * 32 + np.arange(
        N, dtype=np.int32)[None, :]
    s2 = np.where(mask, np.float32(0.0), scores)
    q = np.rint(np.clip((s2 + 8.0) * 1024.0, 0.0, 16256.0)).astype(np.int64)
    v = ((2047 - key).astype(np.int64) << 14) + q
    return v.astype(np.float32)


def _run(scores, ranks, mask, **run_kwargs):
    from concourse.bass_utils import run_bass_kernel_spmd

    nc = _get_program()
    v = np.ascontiguousarray(_pack(scores, ranks, mask))

    in_maps = []
    for c in range(NCORES):
        lo, hi = c * B_CORE, (c + 1) * B_CORE
        in_maps.append({"packed": v[lo:hi]})
    res = run_bass_kernel_spmd(nc, in_maps, core_ids=list(range(NCORES)), **run_kwargs)
    partials = np.stack([r["partial"] for r in res.results])  # [8, 128, 2]
    loss_sum = partials[:, :, 0].sum(dtype=np.float64)
    cnt = partials[:, :, 1].sum(dtype=np.float64)
    out = np.float32(loss_sum / max(cnt, 1.0))
    return out, res


def kernel(scores, ranks, mask):
    out, _ = _run(scores, ranks, mask)
    return np.asarray(out, dtype=np.float32)


# revision 28
# speedup vs baseline: 1.0461x; 1.0461x over previous
"""Plackett-Luce listwise loss kernel for Trainium2 (Bass/Tile), 8-core data parallel.

Algorithm (per row of 32 items):
  loss_row = sum_k log(T_k) - sum_i s_i*valid_i, where T_k are the suffix sums
  of exp(s) over items sorted by (rank, position) (stable sort, padded last).
  Final: mean over rows with n>=2 of loss_row/n.

Host packs each item into one fp32 value
  V = (2047 - ((rank + 32*mask)*32 + pos))*2^14 + round((s*valid + 8)*2^10)
so the device receives a single [B,32] fp32 tensor (inverted key: valid items
carry bit 24, padded do not, and ascending sort puts padded first). Device:
Batcher odd-even merge sort ASCENDING per 32-item segment, int-convert + AND
0x1003FFF (score field q plus the validity bit), ACT exp with bias
-16392 = -8 - 2^24/1024 (valid -> exp(s) exactly, padded -> exp(~-16384) = 0),
gated prefix scan for the suffix sums, ACT ln with +1e-12 bias (padded lanes
scan to exactly 0 -> ln(1e-12) = C0, corrected per row), and two per-row
reductions: sum(ln T) and the int32 sum(u3) whose bits >=24 count valid items
and low bits hold the score sum. Two chains of j=128 segments/partition sort
in stage-lockstep (each chain's ACT copy-backs hide under the other's DVE
compare-exchanges); their decodes then run with pieces alternating between
chains so ACT and DVE dovetail in the tail. Each core reduces to a [128, 2]
partial; host sums and divides.
"""

import sys

for _p in ("/opt/trn_rl_repo", "/root/.axon_site/_ro/trn_rl_repo"):
    if _p not in sys.path:
        sys.path.insert(0, _p)

import numpy as np

P = 128
N = 32
NCORES = 8
B = 262144
B_CORE = B // NCORES  # 32768
CHAINS = [128, 128]  # segments/partition per chain (sums to 256)

# Batcher odd-even merge sort, n=32.
# (k, offset, per-segment pattern [[step,count],...], needs_precopy)
SORT_STAGES = [
    (1, 0, [[2, 16]], False),
    (2, 0, [[4, 8], [1, 2]], False),
    (1, 1, [[4, 8]], True),
    (4, 0, [[8, 4], [1, 4]], False),
    (2, 2, [[8, 4], [1, 2]], True),
    (1, 1, [[8, 4], [2, 3]], True),
    (8, 0, [[16, 2], [1, 8]], False),
    (4, 4, [[16, 2], [1, 4]], True),
    (2, 2, [[16, 2], [4, 3], [1, 2]], True),
    (1, 1, [[16, 2], [2, 7]], True),
    (16, 0, [[1, 16]], False),
    (8, 8, [[1, 8]], True),
    (4, 4, [[8, 3], [1, 4]], True),
    (2, 2, [[4, 7], [1, 2]], True),
    (1, 1, [[2, 15]], True),
]

SC_S = float(2 ** 10)
C0 = float(np.log(np.float32(1e-12)))  # HW Ln table value at the bias


def _pattern_ap(bass_mod, tile_ap, off, dims, j):
    """AP over a [P, j*32] tile selecting `dims` within each 32-item segment."""
    base = tile_ap
    pdim = base.ap[0]
    if dims[0][0] * dims[0][1] == N:
        free = [[dims[0][0], dims[0][1] * j]] + [list(d) for d in dims[1:]]
    else:
        free = [[N, j]] + [list(d) for d in dims]
    return bass_mod.AP(tensor=base.tensor, offset=base.offset + off, ap=[list(pdim)] + free)


def build_program(b_core=B_CORE, chains=CHAINS):
    import concourse.bass as bass
    import concourse.bacc as bacc
    import concourse.tile as tile
    from concourse import mybir

    assert sum(chains) == b_core // P

    nc = bacc.Bacc("TRN2")
    v_d = nc.dram_tensor("packed", [b_core, N], mybir.dt.float32, kind="ExternalInput")
    o_d = nc.dram_tensor("partial", [P, 2], mybir.dt.float32, kind="ExternalOutput")

    op = mybir.AluOpType
    act = mybir.ActivationFunctionType

    with tile.TileContext(nc) as tc:
        with (
            tc.tile_pool(name="singles", bufs=1) as singles,
            tc.tile_pool(name="stream", bufs=1) as stream,
            tc.tile_pool(name="deep", bufs=1) as deep,
        ):
            fmax = max(chains) * N
            _pre = {}

            def _preload(chains_):
                jj0 = 0
                for ci, jj in enumerate(chains_):
                    r0 = jj0 * P
                    ff = jj * N
                    v_a = deep.tile([P, ff], mybir.dt.float32, name=f"va{jj0}")
                    if ci == 0:
                        # first chain split across two queue slots: lands ~2x
                        # sooner, so the first sort stage starts earlier
                        hj = jj // 2
                        nc.sync.dma_start(
                            out=v_a[:, 0:hj * N],
                            in_=v_d[r0:r0 + P * hj, :].rearrange(
                                "(p j) n -> p (j n)", p=P))
                        nc.sync.dma_start(
                            out=v_a[:, hj * N:],
                            in_=v_d[r0 + P * hj:r0 + P * jj, :].rearrange(
                                "(p j) n -> p (j n)", p=P))
                    else:
                        nc.sync.dma_start(
                            out=v_a[:],
                            in_=v_d[r0:r0 + P * jj, :].rearrange("(p j) n -> p (j n)", p=P))
                    _pre[jj0] = v_a
                    jj0 += jj

            _preload(chains)
            gate = singles.tile([P, fmax], mybir.dt.float32)
            nc.vector.memset(gate[:], 1.0)
            g3 = gate[:].rearrange("p (j n) -> p j n", n=N)
            nc.vector.memset(g3[:, :, 0:1], 0.0)
            b_exp = singles.tile([P, 1], mybir.dt.float32)
            nc.vector.memset(b_exp[:], -16392.0)
            b_ln = singles.tile([P, 1], mybir.dt.float32)
            nc.vector.memset(b_ln[:], 1e-12)

            js = b_core // P
            pr_all = singles.tile([P, js], mybir.dt.float32)
            use_all = singles.tile([P, js], mybir.dt.float32)

            def load(j0, j):
                r0 = j0 * P
                f = j * N
                v_a = _pre[j0]
                v_b = deep.tile([P, f], mybir.dt.float32, name=f"vb{j0}")
                scratch = deep.tile([P, f // 2], mybir.dt.float32, name=f"sc{j0}")
                return [v_a, v_b, scratch, j, j0]

            def emit_stage(st, stage):
                (k, off, dims, precopy) = stage
                cur, oth, scratch, j, _ = st
                lo_i = _pattern_ap(bass, cur[:], off, dims, j)
                hi_i = _pattern_ap(bass, cur[:], off + k, dims, j)
                if precopy:
                    npair = j
                    for d in dims:
                        npair *= d[1]
                    sc = scratch[:, 0:npair]
                    nc.vector.tensor_tensor(out=sc, in0=lo_i, in1=hi_i, op=op.min)
                    nc.vector.tensor_tensor(out=hi_i, in0=lo_i, in1=hi_i, op=op.max)
                    nc.scalar.copy(out=lo_i, in_=sc)
                else:
                    lo_o = _pattern_ap(bass, oth[:], off, dims, j)
                    hi_o = _pattern_ap(bass, oth[:], off + k, dims, j)
                    nc.vector.tensor_tensor(out=lo_o, in0=lo_i, in1=hi_i, op=op.min)
                    nc.vector.tensor_tensor(out=hi_o, in0=lo_i, in1=hi_i, op=op.max)
                    st[0], st[1] = oth, cur

            def decode_pieces(st):
                """Decode steps for a sorted chain, yielded between the next
                chain's sort stages so DVE work interleaves and ACT overlaps.
                Elementwise work is chunked (32 segments at a time) so no
                single ACT op can stall the next chain's precopy copy-backs."""
                v_s = st[0]
                j, j0 = st[3], st[4]
                f = j * N
                ctx = {}
                CH = 128  # segments per chunk
                chunks = [(c, min(c + CH, j)) for c in range(0, j, CH)]

                def mk_convert(c0, c1):
                    def p():
                        if "vi" not in ctx:
                            ctx["vi"] = stream.tile([P, f], mybir.dt.int32, name=f"vi{j0}")
                        nc.scalar.copy(out=ctx["vi"][:, c0 * N:c1 * N],
                                       in_=v_s[:, c0 * N:c1 * N])
                    return p

                def mk_and(c0, c1):
                    def p():
                        vi = ctx["vi"]
                        sl = vi[:, c0 * N:c1 * N]
                        nc.vector.tensor_scalar(out=sl, in0=sl, scalar1=0x1003FFF,
                                                scalar2=None, op0=op.bitwise_and)
                    return p

                def mk_usum(c0, c1):
                    def p():
                        if "usum" not in ctx:
                            ctx["usum"] = singles.tile([P, j], mybir.dt.int32, name=f"usum{j0}")
                        with nc.allow_low_precision("int32 reduce exact"):
                            nc.vector.tensor_reduce(
                                out=ctx["usum"][:, c0:c1],
                                in_=ctx["vi"][:, c0 * N:c1 * N].rearrange(
                                    "p (j n) -> p j n", n=N),
                                axis=mybir.AxisListType.X, op=op.add)
                    return p

                def mk_exp(c0, c1):
                    def p():
                        if "e" not in ctx:
                            ctx["e"] = stream.tile([P, f], mybir.dt.float32, name=f"e{j0}")
                        nc.scalar.activation(out=ctx["e"][:, c0 * N:c1 * N],
                                             in_=ctx["vi"][:, c0 * N:c1 * N],
                                             func=act.Exp, bias=b_exp[:],
                                             scale=1.0 / SC_S)
                    return p

                def mk_scan(c0, c1):
                    def p():
                        if "t" not in ctx:
                            ctx["t"] = stream.tile([P, f], mybir.dt.float32, name=f"t{j0}")
                        fc = (c1 - c0) * N
                        nc.vector.tensor_tensor_scan(
                            out=ctx["t"][:, c0 * N:c1 * N], data0=gate[:, 0:fc],
                            data1=ctx["e"][:, c0 * N:c1 * N],
                            initial=0.0, op0=op.mult, op1=op.add)
                    return p

                def mk_ln(c0, c1):
                    def p():
                        sl = ctx["t"][:, c0 * N:c1 * N]
                        nc.scalar.activation(out=sl, in_=sl, func=act.Ln,
                                             bias=b_ln[:], scale=1.0)
                    return p

                def mk_lsum(c0, c1):
                    def p():
                        if "lsum" not in ctx:
                            ctx["lsum"] = singles.tile([P, j], mybir.dt.float32, name=f"lsum{j0}")
                        nc.vector.tensor_reduce(
                            out=ctx["lsum"][:, c0:c1],
                            in_=ctx["t"][:, c0 * N:c1 * N].rearrange(
                                "p (j n) -> p j n", n=N),
                            axis=mybir.AxisListType.X, op=op.add)
                    return p

                def p_weights():
                    # usum = sum(q) + 2^24*n, 0 <= sum(q) < 2^20
                    usum = ctx["usum"]
                    n_i = singles.tile([P, j], mybir.dt.int32)
                    nc.vector.tensor_scalar(out=n_i[:], in0=usum[:], scalar1=24,
                                            scalar2=None, op0=op.arith_shift_right)
                    sq_i = singles.tile([P, j], mybir.dt.int32)
                    nc.vector.tensor_scalar(out=sq_i[:], in0=usum[:],
                                            scalar1=0xFFFFFF, scalar2=None,
                                            op0=op.bitwise_and)
                    n_t = singles.tile([P, j], mybir.dt.float32)
                    nc.vector.tensor_copy(out=n_t[:], in_=n_i[:])
                    # m = svr - C0*npad = sq/1024 - 256 - C0*(32 - n)
                    svr = singles.tile([P, j], mybir.dt.float32)
                    with nc.allow_low_precision("values < 2^20, exact in fp32"):
                        nc.vector.tensor_scalar(out=svr[:], in0=sq_i[:],
                                                scalar1=1.0 / SC_S,
                                                scalar2=-256.0 + 32.0 * C0,
                                                op0=op.mult, op1=op.add)
                    m_t = singles.tile([P, j], mybir.dt.float32)
                    nc.vector.scalar_tensor_tensor(
                        out=m_t[:], in0=n_t[:], scalar=-C0, in1=svr[:],
                        op0=op.mult, op1=op.add)
                    ctx["m"] = m_t
                    # weight = (n>=2)/max(n,1); reciprocal on ACT
                    nmx = singles.tile([P, j], mybir.dt.float32)
                    nc.vector.tensor_scalar_max(nmx[:], n_t[:], 1.0)
                    wrec = singles.tile([P, j], mybir.dt.float32)
                    nc.vector.reciprocal(wrec[:], nmx[:])
                    use = use_all[:, j0:j0 + j]
                    nc.vector.tensor_single_scalar(out=use, in_=n_t[:], scalar=2.0,
                                                   op=op.is_ge)
                    w3 = singles.tile([P, j], mybir.dt.float32)
                    nc.vector.tensor_tensor(out=w3[:], in0=wrec[:], in1=use,
                                            op=op.mult)
                    ctx["w3"] = w3

                def p_final():
                    # pr = (lsum - m) * w3
                    d_t = singles.tile([P, j], mybir.dt.float32)
                    nc.vector.tensor_sub(d_t[:], ctx["lsum"][:], ctx["m"][:])
                    nc.vector.tensor_tensor(out=pr_all[:, j0:j0 + j], in0=d_t[:],
                                            in1=ctx["w3"][:], op=op.mult)

                pieces = []
                for c0, c1 in chunks:
                    pieces += [mk_convert(c0, c1), mk_and(c0, c1),
                               mk_usum(c0, c1), mk_exp(c0, c1),
                               mk_scan(c0, c1), mk_ln(c0, c1), mk_lsum(c0, c1)]
                pieces += [p_weights, p_final]
                return pieces

            # lockstep sorts of the two chains (ACT copy-backs of one chain
            # overlap the other's DVE stages), then their decodes with pieces
            # alternating so ACT and DVE dovetail in the tail
            sts = []
            j0 = 0
            for j in chains:
                sts.append(load(j0, j))
                j0 += j
            plists = None
            for si, stage in enumerate(SORT_STAGES):
                last = si == len(SORT_STAGES) - 1
                for ci, st in enumerate(sts):
                    emit_stage(st, stage)
                    if last and ci == 0:
                        plists = [decode_pieces(s) for s in sts]
                        plists[0].pop(0)()
            k = 0
            while any(plists):
                for pl in plists:
                    if pl:
                        pl.pop(0)()

            out_t = singles.tile([P, 2], mybir.dt.float32)
            nc.vector.tensor_reduce(out=out_t[:, 0:1], in_=pr_all[:],
                                    axis=mybir.AxisListType.X, op=op.add)
            nc.vector.tensor_reduce(out=out_t[:, 1:2], in_=use_all[:],
                                    axis=mybir.AxisListType.X, op=op.add)
            nc.sync.dma_start(out=o_d[:], in_=out_t[:])

    nc.finalize()
    return nc


_CACHED = {}


def _get_program():
    if "nc" not in _CACHED:
        _CACHED["nc"] = build_program()
    return _CACHED["nc"]


def _pack(scores, ranks, mask):
    scores = np.asarray(scores, dtype=np.float32)
    ranks = np.asarray(ranks)
    mask = np.asarray(mask).astype(bool)
    key = (ranks.astype(np.int32) + 32 * mask.astype(np.int32)) * 32 + np.arange(
        N, dtype=np.int32)[None, :]
    s2 = np.where(mask, np.float32(0.0), scores)
    q = np.rint(np.clip((s2 + 8.0) * 1024.0, 0.0, 16256.0)).astype(np.int64)
    v = ((2047 - key).astype(np.int64) << 14) + q
    return v.astype(np.float32)


def _run(scores, ranks, mask, **run_kwargs):
    from concourse.bass_utils import run_bass_kernel_spmd

    nc = _get_program()
    v = np.ascontiguousarray(_pack(scores, ranks, mask))

    in_maps = []
    for c in range(NCORES):
        lo, hi = c * B_CORE, (c + 1) * B_CORE
        in_maps.append({"packed": v[lo:hi]})
    res = run_bass_kernel_spmd(nc, in_maps, core_ids=list(range(NCORES)), **run_kwargs)
    partials = np.stack([r["partial"] for r in res.results])  # [8, 128, 2]
    loss_sum = partials[:, :, 0].sum(dtype=np.float64)
    cnt = partials[:, :, 1].sum(dtype=np.float64)
    out = np.float32(loss_sum / max(cnt, 1.0))
    return out, res


def kernel(scores, ranks, mask):
    out, _ = _run(scores, ranks, mask)
    return np.asarray(out, dtype=np.float32)
